# revision 1
# baseline (speedup 1.0000x reference)
import math

import jax
import jax.numpy as jnp
import numpy as np

C, AC, H, MID, NTR, ITERS = 128, 32, 4, 128, 4, 2
N, B = 256, 1


def _ln(x, g, b):
    mu = jnp.mean(x, axis=-1, keepdims=True)
    var = jnp.mean(jnp.square(x - mu), axis=-1, keepdims=True)
    return (x - mu) * jax.lax.rsqrt(var + 1e-5) * g + b


def _tri_attn(x, mask, p, starting):
    x = _ln(x, p['ln_g'], p['ln_b'])
    sh = x.shape[:-1] + (AC, H)
    q = (x @ p['wq']).reshape(sh)
    k = (x @ p['wk']).reshape(sh)
    v = (x @ p['wv']).reshape(sh)
    bias = x @ p['wb']
    g = jax.nn.sigmoid((x @ p['wg'] + p['bg']).reshape(sh))
    scale = 1.0 / math.sqrt(AC)
    if starting:
        w = jnp.einsum('bijch,bikch->bijkh', q, k) * scale + jnp.swapaxes(bias, 1, 2)[:, None]
        w = (w + 100.0) * mask[:, :, None, :, None] - 100.0
        w = jax.nn.softmax(w, axis=-2)
        o = jnp.einsum('bijkh,bikch->bijch', w, v) * g
    else:
        w = jnp.einsum('bijch,bkjch->bijkh', q, k) * scale + bias[:, :, None]
        w = (w + 100.0) * jnp.swapaxes(mask, 1, 2)[:, None, :, :, None] - 100.0
        w = jax.nn.softmax(w, axis=-2)
        o = jnp.einsum('bijkh,bkjch->bijch', w, v)
        o = o * g
    o = o.reshape(o.shape[:-2] + (AC * H,)) @ p['wo'] + p['bo']
    return o * mask[..., None]


def _tri_mul(x, mask, p, eq):
    x = _ln(x, p['ln_g'], p['ln_b'])
    a = (x @ p['wi'] + p['bi']) * jax.nn.sigmoid(x @ p['wis'] + p['bis'])
    bb = (x @ p['wj'] + p['bj']) * jax.nn.sigmoid(x @ p['wjs'] + p['bjs'])
    a = a * mask[..., None]
    bb = bb * mask[..., None]
    out = jnp.einsum(eq, a, bb)
    out = _ln(out, p['ln2_g'], p['ln2_b'])
    out = out @ p['wp'] + p['bp']
    out = out * jax.nn.sigmoid(x @ p['ws'] + p['bs'])
    return out * mask[..., None]


def _transition(x, p):
    x = _ln(x, p['ln_g'], p['ln_b'])
    x = jax.nn.relu(x @ p['w1'] + p['b1'])
    return x @ p['w2'] + p['b2']


def _stack(x2d, mask, params):
    for lp in params['layers']:
        x2d = x2d + _tri_attn(x2d, mask, lp['tas'], True)
        x2d = x2d + _tri_attn(x2d, mask, lp['tae'], False)
        x2d = x2d + _tri_mul(x2d, mask, lp['tmo'], 'bikc,bjkc->bijc')
        x2d = x2d + _tri_mul(x2d, mask, lp['tmi'], 'bkic,bkjc->bijc')
        x2d = x2d + _transition(x2d, lp['pt'])
        x2d = x2d * mask[..., None]
    x2d = _ln(x2d, params['ln_g'], params['ln_b'])
    return x2d * mask[..., None]


_JIT_CACHE = {}


def _get_sharded_fn():
    """Build a jitted, 8-way i-sharded version of the stack."""
    if 'fn' in _JIT_CACHE:
        return _JIT_CACHE['fn']

    from jax.sharding import Mesh, NamedSharding, PartitionSpec as P

    devs = jax.devices()
    n = 8 if len(devs) >= 8 else len(devs)
    mesh = Mesh(np.array(devs[:n]).reshape(n), ('i',))

    xsh = NamedSharding(mesh, P(None, 'i', None, None))   # shard pair dim i
    msh = NamedSharding(mesh, P())                        # replicate mask
    psh = NamedSharding(mesh, P())                        # replicate params

    def fn(x2d, mask, params):
        return _stack(x2d, mask, params)

    jfn = jax.jit(
        fn,
        in_shardings=(xsh, msh, psh),
        out_shardings=NamedSharding(mesh, P(None, 'i', None, None)),
    )
    _JIT_CACHE['fn'] = (jfn, xsh, msh, psh)
    return _JIT_CACHE['fn']


def kernel(x2d, mask, params):
    x2d = jnp.asarray(np.asarray(x2d), dtype=jnp.float32)
    mask = jnp.asarray(np.asarray(mask), dtype=jnp.float32)
    params = jax.tree_util.tree_map(lambda a: jnp.asarray(np.asarray(a), jnp.float32), params)
    try:
        jfn, xsh, msh, psh = _get_sharded_fn()
        x_d = jax.device_put(x2d, xsh)
        m_d = jax.device_put(mask, msh)
        p_d = jax.device_put(params, psh)
        out = jfn(x_d, m_d, p_d)
        out = jax.block_until_ready(out)
        return np.asarray(out)
    except Exception:
        # fall back to single-device execution
        out = jax.jit(_stack)(x2d, mask, params)
        return np.asarray(jax.block_until_ready(out))
